# revision 9
# baseline (speedup 1.0000x reference)
"""Trainium2 Bass/Tile kernel for the GatedNode2Edge op.

Computes, for emb (B,C,N), th12_* (E,C), th5_* (E,):
    t_k  = th12_k @ emb[b]                      (E,N)
    m_k  = max(t_k[:,i], t_k[:,j]) pairwise     (E,N,N)
    adj  = relu(2*m_1 + th5_1*I)
    gate = sigmoid(relu(2*m_2 + th5_2*I))
    out  = adj * gate                           (B,E,N,N)

Sharding: the 64 (b,e) channels are split 8-per-core across 8 NeuronCores.

Math restructuring (off-diagonal):
    relu(2*max(a,b)) = max(2*relu(a), 2*relu(b))           (relu monotone)
    sigmoid(max(x,y)) = max(sigmoid(x), sigmoid(y))        (sigmoid monotone)
so with row vectors v = relu(t'), g = sigmoid(relu(t')) for t' = 2t:
    out[i,j] = max(v_i, v_j) * max(g_i, g_j)
and the true diagonal out[i,i] = relu(t'_1+th5_1)*sigmoid(relu(t'_2+th5_2))
is folded into the SAME single DVE pass per [128, N] output tile via a
stream-index select:
    out = select(Idx == Latch(Src0)+imm2, Latch(Src1), max(Src0,C0)*max(Src1,C1))
Src0/Src1 are per-channel [128, 8+N] tiles: 8 header columns followed by the
v/g row broadcast across partitions. Header col r of Src0 holds the partition
index (iota); header col r of Src1 holds the true-diagonal value for row-block
r. Tile r streams cols [r:8+N]; latch-init consumes the first stream element
(header r) into the swap flops, the remaining L-1 elements produce the output,
and imm2 = r*128+7-r positions the diagonal. The row broadcast is done by bf16
one-hot selector matmuls on the PE (K=8) into PSUM; the scalar engine copies
PSUM->SBUF. Column-layout operands (vcol/gcol/dcol) are exact fp32 from one
K=65 matmul per 128-node block (emb extended with a ones row, weights packed
[2w1|2w1+th5|2w2|2w2+th5]), so the diagonal is exact and only the j-side of
the off-diagonal max sees bf16 rounding (~3e-3 worst-case vs 2e-2 budget).
"""

import sys
import types

import numpy as np

B, C, N, E = 2, 64, 1024, 32
NCORES = 8
EPC = B * E // NCORES  # 8 channels per core
P = 128
NB = N // P  # 8 row blocks
HW = 8 + N  # header columns + row width
CX = C + 1  # contraction extended with the ones row

_CACHE = {}


def _ensure_hook_shim():
    """Make trace=True safe even when antenv.axon_hooks is absent."""
    try:
        import antenv.axon_hooks  # noqa: F401
    except ImportError:
        mod = types.ModuleType("antenv.axon_hooks")
        mod.get_axon_ntff_profile_hook = lambda: None
        mod.set_axon_ntff_profile_hook = lambda h: None
        sys.modules["antenv.axon_hooks"] = mod


def _register_gated_maxmul_diag():
    """Register the fused out = select(diag, dtrue, max(in0,s0)*max(in1,s1))
    custom DVE op. The diagonal stream position is Latch(Src0)+imm2 (partition
    index from Src0's header plus a per-call immediate); the diagonal value is
    Latch(Src1) (Src1's header)."""
    import concourse.dve_ops as dve_ops
    from concourse.dve_ops import DveOp, OPS, has_src1
    from concourse.dve_spec import (
        C0, C1, C2, AluOp, Bin, Idx, Latch, Spec, Src0, Src1, eq, lower, maxx,
        select,
    )
    from concourse.dve_uop import DveOpSpec

    for op in OPS:
        if op.name == "GATED_MAXMUL_DIAG_ANT":
            return op

    def _ref(in0, in1, s0, s1, imm2):
        # Latch-init consumes element 0 of BOTH sources (both are latched);
        # the steady state then streams elements 1..L-1, with Idx starting
        # at 0 there. Output length is L-1.
        S = in0.shape[-1] - 1
        k = np.arange(S, dtype=np.float32)[None, :]
        dp = in0[..., 0:1] + imm2
        dv = in1[..., 0:1]
        mm = np.maximum(in0[..., 1:], s0) * np.maximum(in1[..., 1:], s1)
        return np.where(k == dp, dv, mm).astype(np.float32)

    spec = Spec(
        body=select(
            eq(Idx, Latch(Bin(AluOp.ADD, Src0, C2))),
            Latch(Src1),
            maxx(Src0, C0) * maxx(Src1, C1),
        ),
        reference=_ref,
    )
    op = DveOp("GATED_MAXMUL_DIAG_ANT", spec, subdim=False, uops_sha={})
    OPS.append(op)
    # Rebuild the registry views that were snapshotted at import time.
    dve_ops.CUSTOM_DVE_SPECS[op.name] = op.spec
    opcode = dve_ops._CUSTOM_DVE_ROW_BASE + len(OPS) - 1
    assert opcode < 0x20
    dve_ops._SUB_OPCODE_FOR_NAME[op.name] = opcode
    # Pin the sha self-consistently (computed exactly as compile() does).
    for ver in ("v3", "v4"):
        s = DveOpSpec(
            name=op.name, opcode=opcode, uops=lower(spec, ver=ver),
            rd1_en=has_src1(spec),
        )
        op.uops_sha[ver] = s.sha(ver)
    return op


def _build_program():
    import concourse.bacc as bacc
    import concourse.mybir as mybir
    import concourse.tile as tile

    dt = mybir.dt.float32
    bf = mybir.dt.bfloat16
    AF = mybir.ActivationFunctionType

    gated_op = _register_gated_maxmul_diag()

    nc = bacc.Bacc("TRN2", target_bir_lowering=False, debug=False, num_devices=NCORES)

    emb = nc.declare_dram_parameter("emb", [CX, N], dt, isOutput=False)
    # Packed weights [CX, 32]: [2w1 | 2w1+th5_1 | 2w2 | 2w2+th5_2] with the
    # bias th5 carried on the extra ones-row of the extended contraction.
    wx = nc.declare_dram_parameter("wx", [CX, 4 * EPC], dt, isOutput=False)
    iota8 = nc.declare_dram_parameter("iota8", [P, EPC], dt, isOutput=False)
    sel32 = nc.declare_dram_parameter("sel32", [EPC, N], dt, isOutput=False)
    out = nc.declare_dram_parameter("out", [EPC, N, N], dt, isOutput=True)

    H = N // 2  # matmul moving free-dim limit is 512

    with tile.TileContext(nc, pool_alloc_mode="queue") as tc:
        with (
            tc.tile_pool(name="const", bufs=1) as cpool,
            tc.tile_pool(name="rows", bufs=1) as rpool,
        ):
            sb_wx = cpool.tile([CX, 4 * EPC], dt)
            nc.sync.dma_start(out=sb_wx[:], in_=wx[:])
            # emb extended with a host-provided ones row (row C) for the th5
            # bias trick.
            sb_embx = cpool.tile([CX, N], dt)
            nc.sync.dma_start(out=sb_embx[:], in_=emb[:])
            sb_iota8 = cpool.tile([P, EPC], dt)
            nc.sync.dma_start(out=sb_iota8[:], in_=iota8[:])
            sb_sel32 = cpool.tile([EPC, N], dt)
            nc.sync.dma_start(out=sb_sel32[:], in_=sel32[:])
            # bf16 copies for the row matmuls + replication (only the j-side
            # broadcast of the off-diagonal max sees this rounding).
            sb_sel16 = cpool.tile([EPC, N], bf)
            nc.scalar.copy(sb_sel16[:], sb_sel32[:])
            sb_embx16 = cpool.tile([CX, N], bf)
            nc.scalar.copy(sb_embx16[:], sb_embx[:])
            sb_wx16 = cpool.tile([CX, 2 * EPC], bf)
            nc.scalar.copy(sb_wx16[:, 0:EPC], sb_wx[:, 0:EPC])
            nc.scalar.copy(sb_wx16[:, EPC:2 * EPC], sb_wx[:, 2 * EPC:3 * EPC])

            # Row-layout intermediates (channel on partition, node on free).
            sb_rowv16 = rpool.tile([EPC, N], bf)  # bf16(relu(t1'))
            sb_rowu = rpool.tile([EPC, N], dt)    # relu(t2')
            sb_rowg16 = rpool.tile([EPC, N], bf)  # bf16(sigmoid(relu(t2')))
            # Column layouts: [p, r*EPC + ch] = value at node r*128+p.
            sb_vcol = rpool.tile([P, NB * EPC], dt)
            sb_gcol = rpool.tile([P, NB * EPC], dt)
            sb_dcol = rpool.tile([P, NB * EPC], dt)

            with (
                tc.tile_pool(name="ph1ps", bufs=1, space="PSUM") as p1ps,
                tc.tile_pool(name="colps", bufs=3, space="PSUM") as cps,
                tc.tile_pool(name="colsb", bufs=3) as csb,
            ):
                ps_t1 = p1ps.tile([EPC, N], dt)
                ps_t2 = p1ps.tile([EPC, N], dt)
                # Per-block column matmuls [CX,128].T @ [CX,32] (exact fp32)
                # feeding the column-space values + the exact diagonal.
                # Sigmoids run on ACT; relus/muls on the (head-idle) DVE.
                for r in range(NB):
                    pc = cps.tile([P, 4 * EPC], dt, tag="pc")
                    nc.tensor.matmul(
                        pc[:], lhsT=sb_embx[:, r * P:(r + 1) * P], rhs=sb_wx[:],
                        start=True, stop=True,
                    )
                    cs = slice(r * EPC, (r + 1) * EPC)
                    nc.vector.tensor_relu(sb_vcol[:, cs], pc[:, 0:EPC])
                    ug = csb.tile([P, EPC], dt, tag="ug")
                    nc.vector.tensor_relu(ug[:], pc[:, 2 * EPC:3 * EPC])
                    nc.scalar.activation(sb_gcol[:, cs], ug[:], AF.Sigmoid)
                    d1 = csb.tile([P, EPC], dt, tag="d1")
                    nc.vector.tensor_relu(d1[:], pc[:, EPC:2 * EPC])
                    d2 = csb.tile([P, EPC], dt, tag="d2")
                    nc.vector.tensor_relu(d2[:], pc[:, 3 * EPC:4 * EPC])
                    nc.scalar.activation(d2[:], d2[:], AF.Sigmoid)
                    nc.vector.tensor_mul(sb_dcol[:, cs], d1[:], d2[:])
                # Phase-1 row matmuls in bf16 (they only feed the bf16
                # broadcast path); relus on DVE, sigmoid on ACT.
                for h in range(2):
                    nc.tensor.matmul(
                        ps_t1[:, h * H:(h + 1) * H],
                        lhsT=sb_wx16[:, 0:EPC],
                        rhs=sb_embx16[:, h * H:(h + 1) * H],
                        start=True,
                        stop=True,
                    )
                    nc.tensor.matmul(
                        ps_t2[:, h * H:(h + 1) * H],
                        lhsT=sb_wx16[:, EPC:2 * EPC],
                        rhs=sb_embx16[:, h * H:(h + 1) * H],
                        start=True,
                        stop=True,
                    )
                nc.vector.tensor_relu(sb_rowv16[:], ps_t1[:])
                nc.vector.tensor_relu(sb_rowu[:], ps_t2[:])
                nc.scalar.activation(sb_rowg16[:], sb_rowu[:], AF.Sigmoid)

            with (
                tc.tile_pool(name="mainps", bufs=2, space="PSUM") as mps,
                tc.tile_pool(name="jtiles", bufs=2) as jp,
                tc.tile_pool(name="work", bufs=8) as wp,
            ):
                for ch in range(EPC):
                    # Broadcast row ch across all 128 partitions with a K=8
                    # one-hot selector matmul (bf16, PSUM f32 out).
                    lsel = sb_sel16[:, ch * P:(ch + 1) * P]
                    ps_v = mps.tile([P, N], dt, tag="ps_v")
                    ps_g = mps.tile([P, N], dt, tag="ps_g")
                    for h in range(2):
                        nc.tensor.matmul(
                            ps_v[:, h * H:(h + 1) * H],
                            lhsT=lsel,
                            rhs=sb_rowv16[:, h * H:(h + 1) * H],
                            start=True,
                            stop=True,
                        )
                        nc.tensor.matmul(
                            ps_g[:, h * H:(h + 1) * H],
                            lhsT=lsel,
                            rhs=sb_rowg16[:, h * H:(h + 1) * H],
                            start=True,
                            stop=True,
                        )
                    sb_vj = jp.tile([P, HW], dt, tag="sb_vj")
                    nc.scalar.copy(sb_vj[:, 0:EPC], sb_iota8[:])
                    nc.scalar.copy(sb_vj[:, EPC:HW], ps_v[:])
                    sb_gj = jp.tile([P, HW], dt, tag="sb_gj")
                    nc.scalar.copy(sb_gj[:, 0:EPC], sb_dcol[:, ch::EPC])
                    nc.scalar.copy(sb_gj[:, EPC:HW], ps_g[:])

                    for r in range(NB):
                        cb = r * P
                        ci = r * EPC + ch
                        # Stream = [header r (eaten by latch-init) | junk
                        # headers r+1..7 | v/g row]; output has one fewer
                        # element than the input stream.
                        o = wp.tile([P, HW], dt, tag="o")
                        nc.vector._custom_dve(
                            gated_op,
                            out=o[:, 0:HW - 1 - r],
                            in0=sb_vj[:, r:HW],
                            in1=sb_gj[:, r:HW],
                            s0=sb_vcol[:, ci:ci + 1],
                            s1=sb_gcol[:, ci:ci + 1],
                            imm2=float(cb + EPC - 1 - r),
                        )
                        nc.sync.dma_start(
                            out=out[ch, cb:cb + P, :],
                            in_=o[:, EPC - 1 - r:HW - 1 - r],
                        )

    nc.compile()
    return nc


def _get_program():
    if "nc" not in _CACHE:
        _CACHE["nc"] = _build_program()
    return _CACHE["nc"]


def _make_wx(th12_1, th12_2, th5_1, th5_2, e0):
    """Packed extended weights [CX, 4*EPC] for channel slice e0:e0+EPC."""
    wx = np.zeros((CX, 4 * EPC), dtype=np.float32)
    w1 = 2.0 * th12_1[e0:e0 + EPC].T  # (C, EPC)
    w2 = 2.0 * th12_2[e0:e0 + EPC].T
    wx[0:C, 0:EPC] = w1
    wx[0:C, EPC:2 * EPC] = w1
    wx[C, EPC:2 * EPC] = th5_1[e0:e0 + EPC]
    wx[0:C, 2 * EPC:3 * EPC] = w2
    wx[0:C, 3 * EPC:4 * EPC] = w2
    wx[C, 3 * EPC:4 * EPC] = th5_2[e0:e0 + EPC]
    return wx


def kernel(**inputs):
    _ensure_hook_shim()
    from concourse.bass_utils import run_bass_kernel_spmd

    emb = np.ascontiguousarray(np.asarray(inputs["emb"], dtype=np.float32))
    th12_1 = np.asarray(inputs["th12_1"], dtype=np.float32)
    th12_2 = np.asarray(inputs["th12_2"], dtype=np.float32)
    th5_1 = np.asarray(inputs["th5_1"], dtype=np.float32)
    th5_2 = np.asarray(inputs["th5_2"], dtype=np.float32)
    iota8 = np.broadcast_to(
        np.arange(P, dtype=np.float32)[:, None], (P, EPC)
    ).copy()
    sel32 = np.zeros((EPC, N), dtype=np.float32)
    for ch in range(EPC):
        sel32[ch, ch * P:(ch + 1) * P] = 1.0

    ones_row = np.ones((1, N), dtype=np.float32)
    in_maps = []
    for k in range(NCORES):
        b = k // (NCORES // B)
        e0 = (k % (NCORES // B)) * EPC
        in_maps.append(
            {
                "emb": np.ascontiguousarray(
                    np.concatenate([emb[b], ones_row], axis=0)
                ),
                "wx": _make_wx(th12_1, th12_2, th5_1, th5_2, e0),
                "iota8": iota8,
                "sel32": sel32,
            }
        )

    nc = _get_program()
    res = run_bass_kernel_spmd(nc, in_maps, core_ids=list(range(NCORES)))
    _CACHE["last_result"] = res

    out = np.empty((B, E, N, N), dtype=np.float32)
    for k in range(NCORES):
        b = k // (NCORES // B)
        e0 = (k % (NCORES // B)) * EPC
        out[b, e0:e0 + EPC] = res.results[k]["out"]
    return out
